# revision 1
# baseline (speedup 1.0000x reference)
"""GCNConv (transform + symmetric-norm aggregate + sigmoid) on 8 Trainium2 NeuronCores.

out_i = sigmoid(dinv_i * sum_{j->i} dinv_j*(xW)_j + dinv_i^2*(xW)_i + b),
dinv = 1/sqrt(1 + in_degree).

Device algorithm (SPMD over 8 cores; per-core differences are pure data):
  pass 0: dinv from CSR rowptr diffs (sub / sqrt / reciprocal on device)
  pass A: g = dinv * (x @ W) for all nodes on every core (tiled matmul from a
          host-transposed x; g stored in a partition-tiled HBM layout split into
          4 quarter-tables so gather indices fit in int16)
  pass B: per 128-dst-node tile: dma_gather (Q7 indexed gather) of g[src] rows
          for the tile's dst-bucketed edge list; one-hot S built on-device
          (DVE is_equal of local-dst ids vs an iota row); segment-sum via PE
          matmuls accumulated in PSUM (self-loop = identity one-hot chunk over
          preloaded own rows); sigmoid(dinv*psum + b); store.

Each core's inputs are rotated by its tile offset so the program is address-
uniform: core c sees global node-tile (t + c*nt_core) % nt_pad at position t,
and its own output tiles are always tiles [0, nt_core).

Host side only re-formats data: COO->CSR bucket sort, padding, int16 index
encoding, x transpose + per-core rotation. All arithmetic runs on device.
"""

import sys

for _p in ("/opt/trn_rl_repo", "/root/.axon_site/_ro/trn_rl_repo"):
    if _p not in sys.path:
        sys.path.append(_p)

import numpy as np

import concourse.bacc as bacc
import concourse.bass as bass
import concourse.mybir as mybir
import concourse.tile as tile
from concourse.bass import ts
from concourse.bass_utils import run_bass_kernel_spmd

P = 128
N_CORES = 8
BATCH_A = 8  # node tiles per pass-A iteration
TB_B = 7  # dst tiles per pass-B gather batch
NQ = 4  # quarter tables (int16 index range)

_prog_cache: dict = {}


def _plan(n_nodes: int):
    nt_real = -(-n_nodes // P)
    nt_pad = nt_real
    while nt_pad % N_CORES or (nt_pad // N_CORES) % TB_B or nt_pad % BATCH_A:
        nt_pad += 1
    return nt_real, nt_pad, nt_pad * P, nt_pad // N_CORES


def preprocess(x: np.ndarray, edge_index: np.ndarray, W: np.ndarray, b: np.ndarray):
    n_nodes, hid = x.shape
    out_dim = W.shape[1]
    nt_real, nt_pad, npad, nt_core = _plan(n_nodes)
    qrows = 32 * nt_pad  # rows per quarter table

    src = np.ascontiguousarray(edge_index[0]).astype(np.int64)
    dst = np.ascontiguousarray(edge_index[1]).astype(np.int64)
    e = src.shape[0]

    counts = np.bincount(dst, minlength=npad)
    rowptr = np.zeros(npad + 1, dtype=np.int64)
    np.cumsum(counts, out=rowptr[1:])

    # bucket edges by (dst tile, src quarter), stable
    tile_of = dst // P
    qr_of = (src % P) // 32
    order = np.argsort(tile_of * NQ + qr_of, kind="stable")
    src_s = src[order]
    dst_s = dst[order]
    grp_s = (tile_of * NQ + qr_of)[order]

    grp_counts = np.bincount(grp_s, minlength=nt_pad * NQ)
    jq = int(max(1, -(-int(grp_counts.max()) // P)))  # chunks per (tile, quarter)
    jc = NQ * jq + 1  # chunks per tile incl. own/self-loop chunk
    slot_cap = jq * P

    grp_start = np.zeros(nt_pad * NQ, dtype=np.int64)
    np.cumsum(grp_counts[:-1], out=grp_start[1:])
    pos = np.arange(e, dtype=np.int64) - grp_start[grp_s]
    slot = grp_s * slot_cap + pos

    # per-edge gather info (tile-rotation applied per core later)
    pp_s = (src_s % P) % 32
    tg_s = src_s // P
    # flat [nt_pad, NQ, slot_cap]
    loc_pp = np.zeros(nt_pad * NQ * slot_cap, dtype=np.int64)
    loc_tg = np.zeros(nt_pad * NQ * slot_cap, dtype=np.int64)
    dl_flat = np.full(nt_pad * NQ * slot_cap, -1.0, dtype=np.float32)
    loc_pp[slot] = pp_s
    loc_tg[slot] = tg_s
    dl_flat[slot] = (dst_s - (dst_s // P) * P).astype(np.float32)

    loc_pp3 = loc_pp.reshape(nt_pad, NQ, slot_cap)
    loc_tg3 = loc_tg.reshape(nt_pad, NQ, slot_cap)

    # dl input [P, nt_pad * jc]: tile chunk cc=(qr*jq+j) at col t*jc+cc; own chunk last
    dl4 = dl_flat.reshape(nt_pad, NQ * jq, P)  # [t, cc, p]
    dl_all = np.empty((P, nt_pad, jc), dtype=np.float32)
    dl_all[:, :, : NQ * jq] = dl4.transpose(2, 0, 1)
    dl_all[:, :, NQ * jq] = np.arange(P, dtype=np.float32)[:, None]

    rp = rowptr.astype(np.float32)
    rp0 = rp[:npad].reshape(nt_pad, P).T.copy()
    rp1 = rp[1 : npad + 1].reshape(nt_pad, P).T.copy()

    xT = np.zeros((hid, npad), dtype=np.float32)
    xT[:, :n_nodes] = np.asarray(x, dtype=np.float32).T
    b_bcast = np.broadcast_to(np.asarray(b, np.float32), (P, out_dim)).copy()

    n_call = TB_B * slot_cap  # idxs per dma_gather call
    cols_call = n_call // 16
    nb = nt_core // TB_B

    shared = dict(W=np.asarray(W, np.float32), b_bcast=b_bcast)
    per_core = []
    for c in range(N_CORES):
        t0 = c * nt_core
        xr = np.roll(xT, -t0 * P, axis=1)
        r0 = np.roll(rp0, -t0, axis=1)
        r1 = np.roll(rp1, -t0, axis=1)
        dlc = np.ascontiguousarray(
            dl_all[:, t0 : t0 + nt_core, :].reshape(P, nt_core * jc)
        )
        # int16 locals with rotated tile index
        tg_rot = (loc_tg3[t0 : t0 + nt_core] - t0) % nt_pad  # [nt_core, NQ, slot_cap]
        loc = (loc_pp3[t0 : t0 + nt_core] * nt_pad + tg_rot).astype(np.int16)
        # calls: batch bb covers tiles [bb*TB_B, ...), per quarter: concat tiles' slots
        # -> [nb, NQ, TB_B*slot_cap]
        loc_b = loc.reshape(nb, TB_B, NQ, slot_cap).transpose(0, 2, 1, 3)
        loc_b = loc_b.reshape(nb * NQ, n_call)
        # wrap each call: idx i -> [i%16, i//16]; stack calls on cols; replicate x8
        wrapped = loc_b.reshape(nb * NQ, cols_call, 16).transpose(0, 2, 1)
        idx16 = np.tile(
            wrapped.transpose(1, 0, 2).reshape(16, nb * NQ * cols_call), (8, 1)
        )
        per_core.append(
            dict(
                xT=xr,
                rp0=r0,
                rp1=r1,
                dl=dlc,
                idx16=np.ascontiguousarray(idx16),
            )
        )
    meta = dict(
        n_nodes=n_nodes,
        hid=hid,
        out_dim=out_dim,
        nt_pad=nt_pad,
        npad=npad,
        nt_core=nt_core,
        jq=jq,
        jc=jc,
        qrows=qrows,
    )
    return meta, shared, per_core


def build_program(meta, variant="full"):
    hid, out_dim = meta["hid"], meta["out_dim"]
    nt_pad, nt_core = meta["nt_pad"], meta["nt_core"]
    jq, jc, qrows = meta["jq"], meta["jc"], meta["qrows"]
    npad = meta["npad"]
    f32, i32, i16 = mybir.dt.float32, mybir.dt.int32, mybir.dt.int16

    n_call = TB_B * jq * P
    cols_call = n_call // 16
    nb = nt_core // TB_B

    nc = bacc.Bacc("TRN2", target_bir_lowering=False, debug=False, num_devices=N_CORES)

    xT_d = nc.dram_tensor("xT", [hid, npad], f32, kind="ExternalInput").ap()
    W_d = nc.dram_tensor("W", [hid, out_dim], f32, kind="ExternalInput").ap()
    b_d = nc.dram_tensor("b_bcast", [P, out_dim], f32, kind="ExternalInput").ap()
    rp0_d = nc.dram_tensor("rp0", [P, nt_pad], f32, kind="ExternalInput").ap()
    rp1_d = nc.dram_tensor("rp1", [P, nt_pad], f32, kind="ExternalInput").ap()
    dl_d = nc.dram_tensor("dl", [P, nt_core * jc], f32, kind="ExternalInput").ap()
    idx_d = nc.dram_tensor(
        "idx16", [P, nb * NQ * cols_call], i16, kind="ExternalInput"
    ).ap()
    # g rows: node n=(t*128+p) at row p*nt_pad + t; quarter q = rows of
    # partitions [32q, 32q+32) — a contiguous int16-addressable sub-table
    g_d = nc.dram_tensor("g", [P * nt_pad, out_dim], f32, kind="Internal").ap()
    out_d = nc.dram_tensor("out", [nt_core * P, out_dim], f32, kind="ExternalOutput").ap()

    gw = g_d.rearrange("(p t) d -> p (t d)", p=P)
    gq_d = [g_d[ts(q, 32 * nt_pad), :] for q in range(NQ)]

    with tile.TileContext(nc) as tc:
        with (
            tc.tile_pool(name="const", bufs=1) as const_pool,
            tc.tile_pool(name="work", bufs=3) as work,
            tc.tile_pool(name="gath", bufs=2) as gath_pool,
            tc.tile_pool(name="smat", bufs=3) as smat_pool,
            tc.tile_pool(name="psum", bufs=4, space="PSUM") as psum_pool,
        ):
            # ---- pass 0: constants + dinv ----
            W_sb = const_pool.tile([hid, out_dim], f32)
            nc.sync.dma_start(W_sb[:], W_d[:])
            b_sb = const_pool.tile([P, out_dim], f32)
            nc.sync.dma_start(b_sb[:], b_d[:])

            dinv = const_pool.tile([P, nt_pad], f32)
            r0 = work.tile([P, nt_pad], f32, tag="rp")
            r1 = work.tile([P, nt_pad], f32, tag="rp")
            nc.sync.dma_start(r0[:], rp0_d[:])
            nc.sync.dma_start(r1[:], rp1_d[:])
            deg = work.tile([P, nt_pad], f32, tag="rp")
            nc.vector.scalar_tensor_tensor(
                out=deg[:],
                in0=r1[:],
                scalar=1.0,
                in1=r0[:],
                op0=mybir.AluOpType.add,
                op1=mybir.AluOpType.subtract,
            )
            sq = work.tile([P, nt_pad], f32, tag="rp")
            nc.scalar.activation(sq[:], deg[:], mybir.ActivationFunctionType.Sqrt)
            nc.vector.reciprocal(dinv[:], sq[:])

            iota_i = const_pool.tile([P, P], i32)
            nc.gpsimd.iota(iota_i[:], pattern=[[1, P]], base=0, channel_multiplier=0)
            iota_f = const_pool.tile([P, P], f32)
            nc.vector.tensor_copy(iota_f[:], iota_i[:])

            # ---- pass A: g = dinv * (x @ W) for all node tiles ----
            # own rows (this core's tiles t < nt_core) are captured into SBUF
            # on the way through, saving a 3.2MB HBM re-read in pass B
            own_sb = const_pool.tile([P, nt_core * out_dim], f32)
            own_v = own_sb[:].rearrange("p (t d) -> p t d", d=out_dim)
            for tb in range(nt_pad // BATCH_A):
                xt = work.tile([hid, BATCH_A * P], f32, tag="xT")
                nc.sync.dma_start(xt[:], xT_d[:, ts(tb, BATCH_A * P)])
                hp = psum_pool.tile([P, BATCH_A * out_dim], f32, tag="psA")
                for k in range(BATCH_A):
                    nc.tensor.matmul(
                        out=hp[:, ts(k, out_dim)],
                        lhsT=xt[:, ts(k, P)],
                        rhs=W_sb[:],
                        start=True,
                        stop=True,
                    )
                gt = work.tile([P, BATCH_A, out_dim], f32, tag="gA")
                for k in range(BATCH_A):
                    nc.vector.tensor_scalar_mul(
                        gt[:, k, :],
                        hp[:, ts(k, out_dim)],
                        dinv[:, tb * BATCH_A + k : tb * BATCH_A + k + 1],
                    )
                nc.scalar.dma_start(
                    gw[:, ts(tb, BATCH_A * out_dim)],
                    gt[:].rearrange("p k d -> p (k d)"),
                )
                lo = tb * BATCH_A
                if lo < nt_core:
                    m = min(BATCH_A, nt_core - lo)
                    nc.vector.tensor_copy(own_v[:, lo : lo + m, :], gt[:, :m, :])

            # ---- pass B ----
            if variant == "a":  # timing probe: skip gather/aggregate work
                zt = const_pool.tile([P, out_dim], f32)
                nc.vector.memset(zt[:], 0.5)
                for t in range(nt_core):
                    nc.scalar.dma_start(out_d[ts(t, P), :], zt[:])
                nb_eff = 0
            else:
                nb_eff = nb
            for bb in range(nb_eff):
                idx_sb = work.tile([P, NQ * cols_call], i16, tag="idx")
                nc.sync.dma_start(idx_sb[:], idx_d[:, ts(bb, NQ * cols_call)])
                dlb = work.tile([P, TB_B * jc], f32, tag="dl")
                nc.sync.dma_start(dlb[:], dl_d[:, ts(bb, TB_B * jc)])
                gath = gath_pool.tile([P, NQ * TB_B * jq, out_dim], f32, tag="gath")
                for q in range(NQ):
                    nc.gpsimd.dma_gather(
                        out_ap=gath[:, ts(q, TB_B * jq), :],
                        in_ap=gq_d[q][:],
                        idxs_ap=idx_sb[:, ts(q, cols_call)],
                        num_idxs=n_call,
                        num_idxs_reg=n_call,
                        elem_size=out_dim,
                        # single_packet packs >=n/16 descs per engine packet;
                        # the 64-desc packet ceiling caps it at 1024 idxs
                        single_packet=n_call <= 1024,
                    )
                for k in range(TB_B):
                    t = bb * TB_B + k
                    dlt = dlb[:, ts(k, jc)]
                    S = smat_pool.tile([P, jc * P], f32, tag="smat")
                    nc.vector.tensor_tensor(
                        out=S[:].rearrange("p (j q) -> p j q", j=jc),
                        in0=dlt[:, :, None].to_broadcast([P, jc, P]),
                        in1=iota_f[:, None, :].to_broadcast([P, jc, P]),
                        op=mybir.AluOpType.is_equal,
                    )
                    op = psum_pool.tile([P, out_dim], f32, tag="psB")
                    for cc in range(jc):
                        if cc < NQ * jq:
                            q, j = divmod(cc, jq)
                            rhs = gath[:, q * TB_B * jq + k * jq + j, :]
                        else:
                            rhs = own_sb[:, ts(t, out_dim)]
                        nc.tensor.matmul(
                            out=op[:],
                            lhsT=S[:, ts(cc, P)],
                            rhs=rhs,
                            start=(cc == 0),
                            stop=(cc == jc - 1),
                        )
                    ot = work.tile([P, out_dim], f32, tag="outt")
                    nc.vector.scalar_tensor_tensor(
                        out=ot[:],
                        in0=op[:],
                        scalar=dinv[:, t : t + 1],
                        in1=b_sb[:],
                        op0=mybir.AluOpType.mult,
                        op1=mybir.AluOpType.add,
                    )
                    osig = work.tile([P, out_dim], f32, tag="osig")
                    nc.scalar.activation(
                        osig[:], ot[:], mybir.ActivationFunctionType.Sigmoid
                    )
                    nc.scalar.dma_start(out_d[ts(t, P), :], osig[:])

    nc.compile()
    return nc


def _get_program(meta):
    key = tuple(sorted((k, v) for k, v in meta.items()))
    if key not in _prog_cache:
        _prog_cache[key] = build_program(meta)
    return _prog_cache[key]


def make_in_maps(meta, shared, per_core):
    return [dict(shared, **per_core[c]) for c in range(N_CORES)]


def kernel(x, edge_index, W, b) -> np.ndarray:
    x = np.asarray(x, np.float32)
    edge_index = np.asarray(edge_index)
    W = np.asarray(W, np.float32)
    b = np.asarray(b, np.float32)

    meta, shared, per_core = preprocess(x, edge_index, W, b)
    nc = _get_program(meta)
    in_maps = make_in_maps(meta, shared, per_core)
    res = run_bass_kernel_spmd(nc, in_maps, core_ids=list(range(N_CORES)))
    outs = [res.results[c]["out"] for c in range(N_CORES)]
    full = np.concatenate(outs, axis=0)
    return full[: meta["n_nodes"]]



# revision 7
# speedup vs baseline: 2.9886x; 2.9886x over previous
"""GCNConv (transform + symmetric-norm aggregate + sigmoid) on 8 Trainium2 NeuronCores.

out_i = sigmoid(dinv_i * sum_{j->i} dinv_j*(xW)_j + dinv_i^2*(xW)_i + b),
dinv = 1/sqrt(1 + in_degree).

Device algorithm (SPMD over 8 cores; per-core differences are pure data):
  pass 0: dinv from CSR rowptr diffs (sub / sqrt / reciprocal on device)
  pass A: g = dinv * (x @ W) for all nodes on every core (tiled matmul from a
          host-transposed x; g stored in a partition-tiled HBM layout split into
          4 quarter-tables so gather indices fit in int16)
  pass B: per 128-dst-node tile: dma_gather (Q7 indexed gather) of g[src] rows
          for the tile's dst-bucketed edge list; one-hot S built on-device
          (DVE is_equal of local-dst ids vs an iota row); segment-sum via PE
          matmuls accumulated in PSUM (self-loop = identity one-hot chunk over
          preloaded own rows); sigmoid(dinv*psum + b); store.

Each core's inputs are rotated by its tile offset so the program is address-
uniform: core c sees global node-tile (t + c*nt_core) % nt_pad at position t,
and its own output tiles are always tiles [0, nt_core).

Host side only re-formats data: COO->CSR bucket sort, padding, int16 index
encoding, x transpose + per-core rotation. All arithmetic runs on device.
"""

import sys

for _p in ("/opt/trn_rl_repo", "/root/.axon_site/_ro/trn_rl_repo"):
    if _p not in sys.path:
        sys.path.append(_p)

import numpy as np

import concourse.bacc as bacc
import concourse.bass as bass
import concourse.mybir as mybir
import concourse.tile as tile
from concourse.bass import ts
from concourse.bass_utils import run_bass_kernel_spmd

P = 128
N_CORES = 8
BATCH_A = 8  # node tiles per pass-A iteration
TB_B = 7  # dst tiles per pass-B gather batch
NQ = 4  # quarter tables (int16 index range)
NQUEUES = 4  # SWDGE queues to spread gathers across

_prog_cache: dict = {}


def _plan(n_nodes: int):
    nt_real = -(-n_nodes // P)
    nt_pad = nt_real
    while nt_pad % N_CORES or (nt_pad // N_CORES) % TB_B or nt_pad % BATCH_A:
        nt_pad += 1
    return nt_real, nt_pad, nt_pad * P, nt_pad // N_CORES


def preprocess(x: np.ndarray, edge_index: np.ndarray, W: np.ndarray, b: np.ndarray):
    n_nodes, hid = x.shape
    out_dim = W.shape[1]
    nt_real, nt_pad, npad, nt_core = _plan(n_nodes)
    qrows = 32 * nt_pad  # rows per quarter table

    src = np.ascontiguousarray(edge_index[0]).astype(np.int64)
    dst = np.ascontiguousarray(edge_index[1]).astype(np.int64)
    e = src.shape[0]

    counts = np.bincount(dst, minlength=npad)
    rowptr = np.zeros(npad + 1, dtype=np.int64)
    np.cumsum(counts, out=rowptr[1:])

    # bucket edges by (dst tile, src quarter), stable
    tile_of = dst // P
    qr_of = (src % P) // 32
    order = np.argsort(tile_of * NQ + qr_of, kind="stable")
    src_s = src[order]
    dst_s = dst[order]
    grp_s = (tile_of * NQ + qr_of)[order]

    grp_counts = np.bincount(grp_s, minlength=nt_pad * NQ)
    jq = int(max(1, -(-int(grp_counts.max()) // P)))  # chunks per (tile, quarter)
    jc = NQ * jq + 1  # chunks per tile incl. own/self-loop chunk
    slot_cap = jq * P

    grp_start = np.zeros(nt_pad * NQ, dtype=np.int64)
    np.cumsum(grp_counts[:-1], out=grp_start[1:])
    pos = np.arange(e, dtype=np.int64) - grp_start[grp_s]
    slot = grp_s * slot_cap + pos

    # per-edge gather info (tile-rotation applied per core later)
    pp_s = (src_s % P) % 32
    tg_s = src_s // P
    # flat [nt_pad, NQ, slot_cap]
    loc_pp = np.zeros(nt_pad * NQ * slot_cap, dtype=np.int64)
    loc_tg = np.zeros(nt_pad * NQ * slot_cap, dtype=np.int64)
    dl_flat = np.full(nt_pad * NQ * slot_cap, -1.0, dtype=np.float32)
    loc_pp[slot] = pp_s
    loc_tg[slot] = tg_s
    dl_flat[slot] = (dst_s - (dst_s // P) * P).astype(np.float32)

    loc_pp3 = loc_pp.reshape(nt_pad, NQ, slot_cap)
    loc_tg3 = loc_tg.reshape(nt_pad, NQ, slot_cap)

    # dl input [P, nt_pad * jc]: tile chunk cc=(qr*jq+j) at col t*jc+cc; own chunk last
    dl4 = dl_flat.reshape(nt_pad, NQ * jq, P)  # [t, cc, p]
    dl_all = np.empty((P, nt_pad, jc), dtype=np.float32)
    dl_all[:, :, : NQ * jq] = dl4.transpose(2, 0, 1)
    dl_all[:, :, NQ * jq] = np.arange(P, dtype=np.float32)[:, None]

    rp = rowptr.astype(np.float32)
    rp0 = rp[:npad].reshape(nt_pad, P).T.copy()
    rp1 = rp[1 : npad + 1].reshape(nt_pad, P).T.copy()

    xT = np.zeros((hid, npad), dtype=np.float32)
    xT[:, :n_nodes] = np.asarray(x, dtype=np.float32).T
    b_bcast = np.broadcast_to(np.asarray(b, np.float32), (P, out_dim)).copy()

    n_call = TB_B * slot_cap  # idxs per dma_gather call
    cols_call = n_call // 16
    nb = nt_core // TB_B

    shared = dict(W=np.asarray(W, np.float32), b_bcast=b_bcast)
    per_core = []
    for c in range(N_CORES):
        t0 = c * nt_core
        xr = np.roll(xT, -t0 * P, axis=1)
        r0 = np.roll(rp0, -t0, axis=1)
        r1 = np.roll(rp1, -t0, axis=1)
        dlc = np.ascontiguousarray(
            dl_all[:, t0 : t0 + nt_core, :].reshape(P, nt_core * jc)
        )
        # int16 locals with rotated tile index
        tg_rot = (loc_tg3[t0 : t0 + nt_core] - t0) % nt_pad  # [nt_core, NQ, slot_cap]
        loc = (loc_pp3[t0 : t0 + nt_core] * nt_pad + tg_rot).astype(np.int16)
        # calls: batch bb covers tiles [bb*TB_B, ...), per quarter: concat tiles' slots
        # -> [nb, NQ, TB_B*slot_cap]
        loc_b = loc.reshape(nb, TB_B, NQ, slot_cap).transpose(0, 2, 1, 3)
        loc_b = loc_b.reshape(nb * NQ, n_call)
        # wrap each call: idx i -> [i%16, i//16]; stack calls on cols; replicate x8
        wrapped = loc_b.reshape(nb * NQ, cols_call, 16).transpose(0, 2, 1)
        idx16 = np.tile(
            wrapped.transpose(1, 0, 2).reshape(16, nb * NQ * cols_call), (8, 1)
        )
        per_core.append(
            dict(
                xT=xr,
                rp0=r0,
                rp1=r1,
                dl=dlc,
                idx16=np.ascontiguousarray(idx16),
            )
        )
    meta = dict(
        n_nodes=n_nodes,
        hid=hid,
        out_dim=out_dim,
        nt_pad=nt_pad,
        npad=npad,
        nt_core=nt_core,
        jq=jq,
        jc=jc,
        qrows=qrows,
    )
    return meta, shared, per_core


def build_program(meta, variant="full"):
    hid, out_dim = meta["hid"], meta["out_dim"]
    nt_pad, nt_core = meta["nt_pad"], meta["nt_core"]
    jq, jc, qrows = meta["jq"], meta["jc"], meta["qrows"]
    npad = meta["npad"]
    f32, i32, i16 = mybir.dt.float32, mybir.dt.int32, mybir.dt.int16

    n_call = TB_B * jq * P
    cols_call = n_call // 16
    nb = nt_core // TB_B

    nc = bacc.Bacc(
        "TRN2",
        target_bir_lowering=False,
        debug=False,
        num_devices=N_CORES,
        num_swdge_queues=NQUEUES,
    )

    xT_d = nc.dram_tensor("xT", [hid, npad], f32, kind="ExternalInput").ap()
    W_d = nc.dram_tensor("W", [hid, out_dim], f32, kind="ExternalInput").ap()
    b_d = nc.dram_tensor("b_bcast", [P, out_dim], f32, kind="ExternalInput").ap()
    rp0_d = nc.dram_tensor("rp0", [P, nt_pad], f32, kind="ExternalInput").ap()
    rp1_d = nc.dram_tensor("rp1", [P, nt_pad], f32, kind="ExternalInput").ap()
    dl_d = nc.dram_tensor("dl", [P, nt_core * jc], f32, kind="ExternalInput").ap()
    idx_d = nc.dram_tensor(
        "idx16", [P, nb * NQ * cols_call], i16, kind="ExternalInput"
    ).ap()
    # g rows: node n=(t*128+p) at row p*nt_pad + t; quarter q = rows of
    # partitions [32q, 32q+32) — a contiguous int16-addressable sub-table
    g_d = nc.dram_tensor("g", [P * nt_pad, out_dim], f32, kind="Internal").ap()
    out_d = nc.dram_tensor("out", [nt_core * P, out_dim], f32, kind="ExternalOutput").ap()

    gw = g_d.rearrange("(p t) d -> p (t d)", p=P)
    gq_d = [g_d[ts(q, 32 * nt_pad), :] for q in range(NQ)]

    do_a = variant not in ("noop",)
    do_b = variant in ("full", "nogath", "nomm", "noS")
    do_gath = variant in ("full", "nomm", "noS")
    do_smm = variant in ("full", "nogath", "noS")
    do_sbuild = variant in ("full", "nogath")

    with tile.TileContext(nc) as tc:
        with (
            tc.tile_pool(name="const", bufs=1) as const_pool,
            tc.tile_pool(name="work", bufs=3) as work,
            tc.tile_pool(name="gath", bufs=2) as gath_pool,
            tc.tile_pool(name="smat", bufs=3) as smat_pool,
            tc.tile_pool(name="psum", bufs=4, space="PSUM") as psum_pool,
        ):
            # ---- pass 0: constants + dinv ----
            W_sb = const_pool.tile([hid, out_dim], f32)
            nc.sync.dma_start(W_sb[:], W_d[:])
            b_sb = const_pool.tile([P, out_dim], f32)
            nc.sync.dma_start(b_sb[:], b_d[:])

            dinv = const_pool.tile([P, nt_pad], f32)
            r0 = work.tile([P, nt_pad], f32, tag="rp")
            r1 = work.tile([P, nt_pad], f32, tag="rp")
            nc.sync.dma_start(r0[:], rp0_d[:])
            nc.sync.dma_start(r1[:], rp1_d[:])
            deg = work.tile([P, nt_pad], f32, tag="rp")
            nc.vector.scalar_tensor_tensor(
                out=deg[:],
                in0=r1[:],
                scalar=1.0,
                in1=r0[:],
                op0=mybir.AluOpType.add,
                op1=mybir.AluOpType.subtract,
            )
            sq = work.tile([P, nt_pad], f32, tag="rp")
            nc.scalar.activation(sq[:], deg[:], mybir.ActivationFunctionType.Sqrt)
            nc.vector.reciprocal(dinv[:], sq[:])

            iota_i = const_pool.tile([P, P], i32)
            nc.gpsimd.iota(iota_i[:], pattern=[[1, P]], base=0, channel_multiplier=0)
            iota_f = const_pool.tile([P, P], f32)
            nc.vector.tensor_copy(iota_f[:], iota_i[:])

            # ---- pass A: g = dinv * (x @ W) for all node tiles ----
            # own rows (this core's tiles t < nt_core) are captured into SBUF
            # on the way through, saving a 3.2MB HBM re-read in pass B
            own_sb = const_pool.tile([P, nt_core * out_dim], f32)
            own_v = own_sb[:].rearrange("p (t d) -> p t d", d=out_dim)
            if not do_a:
                nc.vector.memset(own_sb[:], 0.25)
            for tb in range(nt_pad // BATCH_A if do_a else 0):
                xt = work.tile([hid, BATCH_A * P], f32, tag="xT")
                nc.sync.dma_start(xt[:], xT_d[:, ts(tb, BATCH_A * P)])
                hp = psum_pool.tile([P, BATCH_A * out_dim], f32, tag="psA")
                for k in range(BATCH_A):
                    nc.tensor.matmul(
                        out=hp[:, ts(k, out_dim)],
                        lhsT=xt[:, ts(k, P)],
                        rhs=W_sb[:],
                        start=True,
                        stop=True,
                    )
                gt = work.tile([P, BATCH_A, out_dim], f32, tag="gA")
                for k in range(BATCH_A):
                    nc.vector.tensor_scalar_mul(
                        gt[:, k, :],
                        hp[:, ts(k, out_dim)],
                        dinv[:, tb * BATCH_A + k : tb * BATCH_A + k + 1],
                    )
                nc.scalar.dma_start(
                    gw[:, ts(tb, BATCH_A * out_dim)],
                    gt[:].rearrange("p k d -> p (k d)"),
                )
                lo = tb * BATCH_A
                if lo < nt_core:
                    m = min(BATCH_A, nt_core - lo)
                    nc.vector.tensor_copy(own_v[:, lo : lo + m, :], gt[:, :m, :])

            # ---- pass B ----
            if not do_b:  # timing probes: emit placeholder output stores
                zt = const_pool.tile([P, out_dim], f32)
                nc.vector.memset(zt[:], 0.5)
                for t in range(nt_core):
                    nc.scalar.dma_start(out_d[ts(t, P), :], zt[:])
            S_const = None
            if do_smm and not do_sbuild:  # "noS" probe: one shared S matrix
                S_const = const_pool.tile([P, jc * P], f32)
                nc.vector.memset(S_const[:], 0.0078125)
            gath_const = None
            if do_b and not do_gath:  # "nogath" probe: static gather buffer
                gath_const = const_pool.tile([P, NQ * TB_B * jq, out_dim], f32)
                nc.vector.memset(gath_const[:], 0.125)
            for bb in range(nb if do_b else 0):
                idx_sb = work.tile([P, NQ * cols_call], i16, tag="idx")
                nc.sync.dma_start(idx_sb[:], idx_d[:, ts(bb, NQ * cols_call)])
                dlb = work.tile([P, TB_B * jc], f32, tag="dl")
                nc.sync.dma_start(dlb[:], dl_d[:, ts(bb, TB_B * jc)])
                if do_gath:
                    gath = gath_pool.tile([P, NQ * TB_B * jq, out_dim], f32, tag="gath")
                    for q in range(NQ):
                        nc.gpsimd.dma_gather(
                            out_ap=gath[:, ts(q, TB_B * jq), :],
                            in_ap=gq_d[q][:],
                            idxs_ap=idx_sb[:, ts(q, cols_call)],
                            num_idxs=n_call,
                            num_idxs_reg=n_call,
                            elem_size=out_dim,
                            # single_packet packs >=n/16 descs per engine packet;
                            # the 64-desc packet ceiling caps it at 1024 idxs
                            single_packet=n_call <= 1024,
                            queue_num=(bb * NQ + q) % NQUEUES,
                        )
                else:
                    gath = gath_const
                for k in range(TB_B):
                    t = bb * TB_B + k
                    dlt = dlb[:, ts(k, jc)]
                    if do_sbuild:
                        S = smat_pool.tile([P, jc * P], f32, tag="smat")
                        nc.vector.tensor_tensor(
                            out=S[:].rearrange("p (j q) -> p j q", j=jc),
                            in0=dlt[:, :, None].to_broadcast([P, jc, P]),
                            in1=iota_f[:, None, :].to_broadcast([P, jc, P]),
                            op=mybir.AluOpType.is_equal,
                        )
                    else:
                        S = S_const
                    if do_smm:
                        op = psum_pool.tile([P, out_dim], f32, tag="psB")
                        for cc in range(jc):
                            if cc < NQ * jq:
                                q, j = divmod(cc, jq)
                                rhs = gath[:, q * TB_B * jq + k * jq + j, :]
                            else:
                                rhs = own_sb[:, ts(t, out_dim)]
                            nc.tensor.matmul(
                                out=op[:],
                                lhsT=S[:, ts(cc, P)],
                                rhs=rhs,
                                start=(cc == 0),
                                stop=(cc == jc - 1),
                            )
                        src_fin = op[:]
                    else:
                        src_fin = own_sb[:, ts(t, out_dim)]
                    ot = work.tile([P, out_dim], f32, tag="outt")
                    nc.vector.scalar_tensor_tensor(
                        out=ot[:],
                        in0=src_fin,
                        scalar=dinv[:, t : t + 1],
                        in1=b_sb[:],
                        op0=mybir.AluOpType.mult,
                        op1=mybir.AluOpType.add,
                    )
                    osig = work.tile([P, out_dim], f32, tag="osig")
                    nc.scalar.activation(
                        osig[:], ot[:], mybir.ActivationFunctionType.Sigmoid
                    )
                    nc.scalar.dma_start(out_d[ts(t, P), :], osig[:])

    nc.compile()
    return nc


def _get_program(meta):
    key = tuple(sorted((k, v) for k, v in meta.items()))
    if key not in _prog_cache:
        _prog_cache[key] = build_program(meta)
    return _prog_cache[key]


def make_in_maps(meta, shared, per_core):
    return [dict(shared, **per_core[c]) for c in range(N_CORES)]


def kernel(x, edge_index, W, b) -> np.ndarray:
    x = np.asarray(x, np.float32)
    edge_index = np.asarray(edge_index)
    W = np.asarray(W, np.float32)
    b = np.asarray(b, np.float32)

    meta, shared, per_core = preprocess(x, edge_index, W, b)
    nc = _get_program(meta)
    in_maps = make_in_maps(meta, shared, per_core)
    res = run_bass_kernel_spmd(nc, in_maps, core_ids=list(range(N_CORES)))
    outs = [res.results[c]["out"] for c in range(N_CORES)]
    full = np.concatenate(outs, axis=0)
    return full[: meta["n_nodes"]]

